# revision 15
# baseline (speedup 1.0000x reference)
"""Trainium2 Bass kernel for a causal dense-transformer attention layer.

Reference computation (b=4, s=2048, d=1024, 16 heads, dh=64):
  qkv = x0 @ W_in ; causal softmax attention ; out = attn @ W_o
  y = LayerNorm(out + x0)   (no affine, eps=1e-5)

Sharding over 8 cores: core = (batch bi = core//2, head-group tp = core%2).
Each core computes QKV projection + attention for its 8 heads of one batch
(tensor parallel over head groups); the output projection's partial sums are
pair-ReduceScattered (4 chunks, one per 512-query sweep, each fired as soon
as its sweep drains so the collective hides under the next sweep), then
residual + LayerNorm run locally on the 1024 rows each core ends up with.

On-chip layout: scores are computed transposed (keys on partitions, queries
on the free axis) so attn @ V needs no transposes; the two heads of a pair
are packed into one 2-bank PSUM tile so a single merged ACT exp covers both
(ACT is the attention-phase bottleneck: (N+352)/1.2ns per activation).
K=64 score matmul pairs row-tile into the PE quadrants; attn@V carries a
trailing ones column per head whose PSUM row yields the softmax denominator.
Diagonal (causal) blocks slice the av matmul + mask columns to skip work.
QKV projection is interleaved into the first attention sweep so the PE's
projection work hides under ACT-bound attention. LayerNorm's rstd uses
exp(-0.5*ln(var+eps)) to stay on the ACT exp/ln table set (no table thrash);
softmax normalization uses the DVE fast-reciprocal approximation.
"""

import os
import sys
from contextlib import ExitStack

import numpy as np

for _p in ("/opt/trn_rl_repo",):
    if os.path.isdir(_p) and _p not in sys.path:
        sys.path.insert(0, _p)

import concourse.bass as bass
import concourse.tile as tile
from concourse import bacc
from concourse import mybir
from concourse.bass_utils import run_bass_kernel_spmd

B, S, D = 4, 2048, 1024
NH, DH = 16, 64
HL = NH // 2          # heads per core
SH = S // 2           # output seq rows per core
NCORES = 8
SCALE = DH ** -0.5    # 0.125
LN_EPS = 1e-5
CHUNK_QC = (0, 2, 1, 3)   # output chunk c covers queries [qc*512, qc*512+512)

F16 = mybir.dt.float16
F32 = mybir.dt.float32
F8 = mybir.dt.float8e4
DR = mybir.MatmulPerfMode.DoubleRow
Exp = mybir.ActivationFunctionType.Exp
Ln = mybir.ActivationFunctionType.Ln


def build_nc():
    nc = bacc.Bacc("TRN2", target_bir_lowering=False, num_devices=NCORES)
    xT = nc.declare_dram_parameter("xT", [D, S], F16, isOutput=False)
    wqk = nc.declare_dram_parameter("wqk", [D, 2 * HL * DH], F16, isOutput=False)
    wv = nc.declare_dram_parameter("wv", [D, HL * DH], F16, isOutput=False)
    wo = nc.declare_dram_parameter("wo", [HL * DH, D], F16, isOutput=False)
    xres = nc.declare_dram_parameter("xres", [SH, D], F16, isOutput=False)
    cmsk = nc.declare_dram_parameter("cmask", [128, 4 * 512], F16, isOutput=False)
    out = nc.declare_dram_parameter("out", [SH, D], F16, isOutput=True)

    with tile.TileContext(nc, num_cores=NCORES) as tc, ExitStack() as top:
        persist = top.enter_context(tc.tile_pool(name="persist", bufs=1))
        # QT rows 0..511 (tiles 0-3, head pair t on tile t), KT rows 512..1023
        qkt = [persist.tile([128, S], F16, name=f"qkt{m}") for m in range(8)]
        # V in (seq-part, head*dh free) orientation, 16 seq tiles
        vsb = [persist.tile([128, HL * (DH + 1)], F16, name=f"vsb{m}") for m in range(16)]
        # attn-out^T (head*dh on partitions, seq free); written unnormalized
        # by the PSUM drains, then normalized in place
        aot = [persist.tile([128, S], F16, name=f"aot{t}") for t in range(4)]
        cm = persist.tile([128, 4 * 512], F16, name="cm")
        ones_bc = persist.tile([65, 64], F32, name="ones_bc")
        nc.vector.memset(ones_bc, 1.0)
        eps_t = persist.tile([128, 1], F32, name="eps_t")
        nc.vector.memset(eps_t, LN_EPS)
        for m in range(16):
            vones = vsb[m].rearrange("p (h c) -> p h c", c=DH + 1)[:, :, DH:DH + 1]
            nc.vector.memset(vones, 1.0)
        nc.sync.dma_start(out=cm, in_=cmsk[:, :])

        # pool creation order is LIFO per memory space: fin pools (live to
        # the end) first, then attention pools, then projection pools last so
        # proj_ctx can close mid-program and free its SBUF/PSUM for fps.
        fin = ExitStack()
        dpool = fin.enter_context(tc.tile_pool(name="dram", bufs=1, space="DRAM"))
        fsb = fin.enter_context(tc.tile_pool(name="fsb", bufs=1))
        lnp = fin.enter_context(tc.tile_pool(name="lnp", bufs=2))
        wos = [fsb.tile([128, D], F16, name=f"wos{k}") for k in range(4)]
        xr = [fsb.tile([128, D], F16, name=f"xr{k}") for k in range(8)]

        attn_ctx = ExitStack()
        adram = attn_ctx.enter_context(tc.tile_pool(name="adram", bufs=2, space="DRAM"))
        asb = attn_ctx.enter_context(tc.tile_pool(name="asb", bufs=4))
        scps = attn_ctx.enter_context(tc.tile_pool(name="scps", bufs=2, space="PSUM"))
        accps = attn_ctx.enter_context(tc.tile_pool(name="accps", bufs=1, space="PSUM"))
        small = attn_ctx.enter_context(tc.tile_pool(name="small", bufs=2))

        # ---- input DMAs: weights first, then x by seq chunks so the
        # projection can start after ~1/4 of x has landed; wo/xres last
        proj_ctx = ExitStack()
        proj_in = proj_ctx.enter_context(tc.tile_pool(name="proj_in", bufs=1))
        pjps = proj_ctx.enter_context(tc.tile_pool(name="pjps", bufs=2, space="PSUM"))
        xt = [proj_in.tile([128, S], F16, name=f"xt{k}") for k in range(8)]
        wqs = [proj_in.tile([128, 2 * HL * DH], F16, name=f"wqs{k}") for k in range(8)]
        wvs = [proj_in.tile([128, HL * DH], F16, name=f"wvs{k}") for k in range(8)]
        for k in range(8):
            nc.sync.dma_start(out=wvs[k], in_=wv[k * 128:(k + 1) * 128, :])
        for k in range(8):
            nc.sync.dma_start(out=xt[k][:, 0:512], in_=xT[k * 128:(k + 1) * 128, 0:512])
        for k in range(8):
            nc.sync.dma_start(out=wqs[k], in_=wqk[k * 128:(k + 1) * 128, :])
        for k in range(8):
            nc.sync.dma_start(out=xt[k][:, 512:2048], in_=xT[k * 128:(k + 1) * 128, 512:2048])
        for k in range(4):
            nc.sync.dma_start(out=wos[k], in_=wo[k * 128:(k + 1) * 128, :])
        for k in range(8):
            nc.sync.dma_start(out=xr[k], in_=xres[k * 128:(k + 1) * 128, :])

        # tiny AllReduce as a cross-core barrier: absorbs NEFF launch skew
        # between pair cores while the input DMAs stream, so the real
        # ReduceScatters later don't each pay a ~30us peer-wait
        bar_in = dpool.tile([2, 4], F16, name="bar_in", bufs=1)
        bar_out = dpool.tile([2, 4], F16, name="bar_out", bufs=1)
        nc.sync.dma_start(out=bar_in.opt(), in_=cmsk[0:2, 0:4])
        nc.gpsimd.collective_compute(
            "AllReduce", mybir.AluOpType.add,
            replica_groups=[[0, 1], [2, 3], [4, 5], [6, 7]],
            ins=[bar_in.opt()], outs=[bar_out.opt()])

        def proj_v(m):
            ps = pjps.tile([128, 512], F32, tag="pj", name="pjv")
            for k in range(8):
                nc.tensor.matmul(ps, xt[k][:, m * 128:(m + 1) * 128], wvs[k],
                                 start=(k == 0), stop=(k == 7))
            vdst = vsb[m].rearrange("p (h c) -> p h c", c=DH + 1)[:, :, 0:DH]
            nc.vector.tensor_copy(vdst, ps.rearrange("p (h c) -> p h c", c=DH))

        def proj_qk(m):
            for q4 in range(4):
                ps = pjps.tile([128, 512], F32, tag="pj", name="pjqk")
                for k in range(8):
                    nc.tensor.matmul(ps, wqs[k][:, m * 128:(m + 1) * 128],
                                     xt[k][:, q4 * 512:(q4 + 1) * 512],
                                     start=(k == 0), stop=(k == 7))
                nc.vector.tensor_copy(qkt[m][:, q4 * 512:(q4 + 1) * 512], ps)

        def attn_chunk(t, qc):
            q_t, k_t = qkt[t], qkt[4 + t]
            nkb = 4 * qc + 4
            qsl = slice(qc * 512, (qc + 1) * 512)
            av0 = accps.tile([65, 512], F32, tag="av0", name="av0")
            av1 = accps.tile([65, 512], F32, tag="av1", name="av1")
            w = DH + 1

            def scores(kb):
                # both heads' scores in one 2-bank PSUM tile: one merged exp.
                # The two K=64 matmuls row-tile into PE quadrants (0,0)/(64,0).
                ksl = slice(kb * 128, (kb + 1) * 128)
                s01 = scps.tile([128, 1024], F32, tag="s01", name="s01")
                nc.tensor.matmul(s01[:, 0:512], k_t[0:64, ksl], q_t[0:64, qsl],
                                 start=True, stop=True)
                nc.tensor.matmul(s01[:, 512:1024], k_t[64:128, ksl], q_t[64:128, qsl],
                                 start=True, stop=True)
                e01 = asb.tile([128, 1024], F16, tag="e01", name="e01", bufs=4)
                nc.scalar.activation(e01, s01, Exp, scale=SCALE)
                r = kb - 4 * qc
                qoff = r * 128 if r > 0 else 0
                if r >= 0:  # diagonal block: zero masked entries (valid cols only)
                    msl = slice(r * 512 + qoff, (r + 1) * 512)
                    nc.vector.tensor_mul(e01[:, qoff:512], e01[:, qoff:512], cm[:, msl])
                    nc.vector.tensor_mul(e01[:, 512 + qoff:1024],
                                         e01[:, 512 + qoff:1024], cm[:, msl])
                return e01, qoff

            def av(kb, e01, qoff, st, sp):
                # attn-out^T accumulation; V carries a trailing ones column,
                # so the softmax denominator accumulates into PSUM row 64.
                # Diagonal blocks only contribute to queries >= qoff.
                nc.tensor.matmul(av0[:, qoff:512], vsb[kb][:, (2 * t) * w:(2 * t + 1) * w],
                                 e01[:, qoff:512], start=st, stop=sp)
                nc.tensor.matmul(av1[:, qoff:512], vsb[kb][:, (2 * t + 1) * w:(2 * t + 2) * w],
                                 e01[:, 512 + qoff:1024], start=st, stop=sp)

            # key blocks in pairs: both blocks' score matmuls (64-row PE mode)
            # run back-to-back, then both av matmuls (128-row mode) — halves
            # the PE tiling-mode switches, which drain the array
            for kb0 in range(0, nkb, 2):
                e_a, qo_a = scores(kb0)
                e_b, qo_b = scores(kb0 + 1)
                av(kb0, e_a, qo_a, kb0 == 0, False)
                av(kb0 + 1, e_b, qo_b, False, kb0 + 1 == nkb - 1)
            # drain PSUM (unnormalized, fp16). Denominator rows are copied to
            # SBUF, then a K=1 ones-matmul replicates each across 64 PSUM
            # partitions (no DRAM roundtrip), fast-reciprocal, normalize.
            d0 = small.tile([65, 512], F32, tag="d0", name="d0")
            d1 = small.tile([65, 512], F32, tag="d1", name="d1")
            nc.vector.tensor_copy(d0[64:65, :], av0[64:65, :])
            nc.vector.tensor_copy(d1[64:65, :], av1[64:65, :])
            nc.vector.tensor_copy(aot[t][0:64, qsl], av0[0:64, :])
            stg = small.tile([64, 512], F16, tag="stg", name="stg", bufs=4)
            nc.vector.tensor_copy(stg, av1[0:64, :])
            nc.sync.dma_start(out=aot[t][64:128, qsl], in_=stg)
            rbps = scps.tile([128, 1024], F32, tag="s01", name="rbps")
            nc.tensor.matmul(rbps[0:64, 0:512], ones_bc[64:65, :], d0[64:65, :],
                             start=True, stop=True)
            nc.tensor.matmul(rbps[64:128, 0:512], ones_bc[64:65, :], d1[64:65, :],
                             start=True, stop=True)
            rb = small.tile([128, 512], F32, tag="rb", name="rb")
            with tc.high_priority(offset=-800):
                nc.vector.reciprocal_approx_fast(rb, rbps[:, 0:512])
                nc.vector.tensor_mul(aot[t][:, qsl], aot[t][:, qsl], rb)

        # fps (PSUM for the output projection) is allocated after proj_ctx
        # closes so it can reuse pjps's banks (8-bank budget).
        fps = None
        # per-pair-chunk fp16 ReduceScatter: chunk c carries output rows
        # [qc*512, qc*512+512); even core keeps the first 256, odd the last.
        rs_in = [dpool.tile([512, D], F16, name=f"rs_in{c}", bufs=1) for c in range(4)]
        rs_out = [dpool.tile([256, D], F16, name=f"rs_out{c}", bufs=1) for c in range(4)]

        rsq_magic = persist.tile([128, 1], mybir.dt.int32, name="rsq_magic")
        nc.vector.memset(rsq_magic.bitcast(F32), np.uint32(0x5F3759DF).view(np.float32).item())

        def _rsqrt_dve(v, out):
            """out = (v+eps)^-0.5 via quake seed + 2 Newton-Raphson steps."""
            ve = lnp.tile([128, 1], F32, tag="rq_ve", name="rq_ve")
            nc.vector.tensor_scalar(out=ve, in0=v, scalar1=LN_EPS, scalar2=None,
                                    op0=mybir.AluOpType.add)
            ri = lnp.tile([128, 1], mybir.dt.int32, tag="rq_ri", name="rq_ri")
            nc.vector.tensor_scalar(out=ri, in0=ve.bitcast(mybir.dt.int32), scalar1=1,
                                    scalar2=None, op0=mybir.AluOpType.logical_shift_right)
            nc.vector.tensor_tensor(out=ri, in0=rsq_magic, in1=ri,
                                    op=mybir.AluOpType.subtract)
            r = ri.bitcast(F32)
            hv = lnp.tile([128, 1], F32, tag="rq_hv", name="rq_hv")
            nc.vector.tensor_scalar(out=hv, in0=ve, scalar1=0.5, scalar2=None,
                                    op0=mybir.AluOpType.mult)
            t1 = lnp.tile([128, 1], F32, tag="rq_t1", name="rq_t1")
            for it in range(2):
                nc.vector.tensor_tensor(out=t1, in0=r, in1=r, op=mybir.AluOpType.mult)
                nc.vector.tensor_tensor(out=t1, in0=t1, in1=hv, op=mybir.AluOpType.mult)
                nc.vector.tensor_scalar(out=t1, in0=t1, scalar1=-1.0, scalar2=1.5,
                                        op0=mybir.AluOpType.mult, op1=mybir.AluOpType.add)
                dst = out if it == 1 else r
                nc.vector.tensor_tensor(out=dst, in0=r, in1=t1, op=mybir.AluOpType.mult)

        def out_chunk(c):
            qc = CHUNK_QC[c]
            for j in range(4):
                g = qc * 4 + j
                pstg = lnp.tile([128, D], F16, tag="pstg", name="pstg")
                po = fps.tile([128, 1024], F32, tag="po", name="po")
                for n2 in range(2):
                    for k in range(4):
                        nc.tensor.matmul(po[:, n2 * 512:(n2 + 1) * 512],
                                         aot[k][:, g * 128:(g + 1) * 128],
                                         wos[k][:, n2 * 512:(n2 + 1) * 512],
                                         start=(k == 0), stop=(k == 3))
                nc.vector.tensor_copy(pstg, po)
                nc.sync.dma_start(out=rs_in[c][j * 128:(j + 1) * 128, :], in_=pstg)
            nc.gpsimd.collective_compute(
                "ReduceScatter", mybir.AluOpType.add,
                replica_groups=[[0, 1], [2, 3], [4, 5], [6, 7]],
                ins=[rs_in[c].opt()], outs=[rs_out[c].opt()])

        def ln_chunk(c):
          with tc.high_priority(offset=-2000):
            for j in range(2):
                m = 2 * c + j
                yin = lnp.tile([128, D], F16, tag="yin", name="yin")
                nc.gpsimd.dma_start(out=yin, in_=rs_out[c][j * 128:(j + 1) * 128, :])
                y = lnp.tile([128, D], F16, tag="y", name="y")
                nc.vector.tensor_add(y, yin, xr[m])
                stats = lnp.tile([128, 2, 6], F32, tag="st", name="st")
                mv = lnp.tile([128, 2], F32, tag="mv", name="mv")
                for sg in range(2):
                    nc.vector.bn_stats(out=stats[:, sg, :], in_=y[:, sg * 512:(sg + 1) * 512])
                nc.vector.bn_aggr(out=mv, in_=stats)
                # rstd = (var+eps)^-0.5 on DVE (magic-constant seed + 2
                # Newton steps) so ACT only ever needs the exp table set
                rstd = lnp.tile([128, 1], F32, tag="rs", name="rs")
                _rsqrt_dve(mv[:, 1:2], rstd)
                ot = lnp.tile([128, D], F16, tag="ot", name="ot")
                nc.vector.tensor_scalar(out=ot, in0=y, scalar1=mv[:, 0:1], scalar2=rstd,
                                        op0=mybir.AluOpType.subtract,
                                        op1=mybir.AluOpType.mult)
                nc.sync.dma_start(out=out[m * 128:(m + 1) * 128, :], in_=ot)

        # ---- schedule: projection interleaved into the first (qc=0) sweep;
        # each RS fires right after its sweep so it hides under the next one.
        for t in range(4):
            proj_v(4 * t)
            proj_v(4 * t + 1)
            proj_v(4 * t + 2)
            proj_v(4 * t + 3)
            proj_qk(t)
            proj_qk(4 + t)
            attn_chunk(t, 0)
        proj_ctx.close()
        fps_ctx = ExitStack()
        fps = fps_ctx.enter_context(tc.tile_pool(name="fps", bufs=1, space="PSUM"))
        out_chunk(0)
        for t in range(4):
            attn_chunk(t, 2)
        out_chunk(1)
        for t in range(4):
            attn_chunk(t, 1)
        out_chunk(2)
        ln_chunk(0)
        for t in range(4):
            attn_chunk(t, 3)
        out_chunk(3)
        ln_chunk(1)
        ln_chunk(2)
        ln_chunk(3)
        fps_ctx.close()
        attn_ctx.close()
        fin.close()
    nc.compile()
    return nc


def _build_cmask():
    k = np.arange(128)[:, None]
    q = np.arange(512)[None, :]
    blocks = [(r * 128 + k <= q).astype(np.float16) for r in range(4)]
    return np.concatenate(blocks, axis=1)


def _dr8(a):
    """[1024, N] -> DoubleRow-interleaved fp8e4m3 [128, 8, N]."""
    import ml_dtypes
    n = a.shape[1]
    return np.ascontiguousarray(
        np.asarray(a, np.float32).reshape(8, 128, n).transpose(1, 0, 2)
    ).astype(ml_dtypes.float8_e4m3)


def _core_rows(half):
    """Absolute query rows this core outputs, in out-tile order."""
    rows = []
    for c in range(4):
        qbase = CHUNK_QC[c] * 512 + half * 256
        rows.append(np.arange(qbase, qbase + 256))
    return np.concatenate(rows)


def _make_in_maps(x0, W_in, W_o):
    x0 = np.asarray(x0, np.float32)
    W_in = np.asarray(W_in, np.float32)
    W_o = np.asarray(W_o, np.float32)
    wo16 = W_o.astype(np.float16)
    cmask = _build_cmask()
    in_maps = []
    for core in range(NCORES):
        bi, half = core // 2, core % 2
        hs = range(half * HL, half * HL + HL)
        wqk = np.concatenate(
            [W_in[:, h * 3 * DH: h * 3 * DH + DH] for h in hs]
            + [W_in[:, h * 3 * DH + DH: h * 3 * DH + 2 * DH] for h in hs], axis=1)
        wv = np.concatenate(
            [W_in[:, h * 3 * DH + 2 * DH: h * 3 * DH + 3 * DH] for h in hs], axis=1)
        rows = _core_rows(half)
        in_maps.append(dict(
            xT=np.ascontiguousarray(x0[bi].T).astype(np.float16),
            wqk=np.ascontiguousarray(wqk).astype(np.float16),
            wv=np.ascontiguousarray(wv).astype(np.float16),
            wo=np.ascontiguousarray(wo16[half * HL * DH:(half + 1) * HL * DH]),
            xres=np.ascontiguousarray(x0[bi][rows]).astype(np.float16),
            cmask=cmask))
    return in_maps


_NC = None


def _run(x0, W_in, W_o, **run_kwargs):
    global _NC
    if _NC is None:
        _NC = build_nc()
    in_maps = _make_in_maps(x0, W_in, W_o)
    return run_bass_kernel_spmd(_NC, in_maps, list(range(NCORES)), **run_kwargs)


def kernel(x0, W_in, W_o, src_mask=None):
    res = _run(x0, W_in, W_o).results
    out = np.empty((B, S, D), np.float32)
    for core in range(NCORES):
        bi, half = core // 2, core % 2
        out[bi][_core_rows(half)] = res[core]["out"].astype(np.float32)
    return out


# revision 17
# speedup vs baseline: 1.0844x; 1.0844x over previous
"""Trainium2 Bass kernel for a causal dense-transformer attention layer.

Reference computation (b=4, s=2048, d=1024, 16 heads, dh=64):
  qkv = x0 @ W_in ; causal softmax attention ; out = attn @ W_o
  y = LayerNorm(out + x0)   (no affine, eps=1e-5)

Sharding over 8 cores: core = (batch bi = core//2, head-group tp = core%2).
Each core computes QKV projection + attention for its 8 heads of one batch
(tensor parallel over head groups); the output projection's partial sums are
pair-ReduceScattered (4 chunks, one per 512-query sweep, each fired as soon
as its sweep drains so the collective hides under the next sweep), then
residual + LayerNorm run locally on the 1024 rows each core ends up with.

On-chip layout: scores are computed transposed (keys on partitions, queries
on the free axis) so attn @ V needs no transposes; the two heads of a pair
are packed into one 2-bank PSUM tile so a single merged ACT exp covers both
(ACT is the attention-phase bottleneck: (N+352)/1.2ns per activation).
K=64 score matmul pairs row-tile into the PE quadrants; attn@V carries a
trailing ones column per head whose PSUM row yields the softmax denominator.
Diagonal (causal) blocks slice the av matmul + mask columns to skip work.
QKV projection is interleaved into the first attention sweep so the PE's
projection work hides under ACT-bound attention. LayerNorm's rstd uses
exp(-0.5*ln(var+eps)) to stay on the ACT exp/ln table set (no table thrash);
softmax normalization uses the DVE fast-reciprocal approximation.
"""

import os
import sys
from contextlib import ExitStack

import numpy as np

for _p in ("/opt/trn_rl_repo",):
    if os.path.isdir(_p) and _p not in sys.path:
        sys.path.insert(0, _p)

import concourse.bass as bass
import concourse.tile as tile
from concourse import bacc
from concourse import mybir
from concourse.bass_utils import run_bass_kernel_spmd

B, S, D = 4, 2048, 1024
NH, DH = 16, 64
HL = NH // 2          # heads per core
SH = S // 2           # output seq rows per core
NCORES = 8
SCALE = DH ** -0.5    # 0.125
LN_EPS = 1e-5
CHUNK_QC = (0, 2, 1, 3)   # output chunk c covers queries [qc*512, qc*512+512)

F16 = mybir.dt.float16
F32 = mybir.dt.float32
F8 = mybir.dt.float8e4
DR = mybir.MatmulPerfMode.DoubleRow
Exp = mybir.ActivationFunctionType.Exp
Ln = mybir.ActivationFunctionType.Ln


def build_nc():
    nc = bacc.Bacc("TRN2", target_bir_lowering=False, num_devices=NCORES)
    xT = nc.declare_dram_parameter("xT", [D, S], F16, isOutput=False)
    wqk = nc.declare_dram_parameter("wqk", [D, 2 * HL * DH], F16, isOutput=False)
    wv = nc.declare_dram_parameter("wv", [D, HL * DH], F16, isOutput=False)
    wo = nc.declare_dram_parameter("wo", [HL * DH, D], F16, isOutput=False)
    xres = nc.declare_dram_parameter("xres", [SH, D], F16, isOutput=False)
    cmsk = nc.declare_dram_parameter("cmask", [128, 4 * 512], F16, isOutput=False)
    out = nc.declare_dram_parameter("out", [SH, D], F16, isOutput=True)

    with tile.TileContext(nc, num_cores=NCORES) as tc, ExitStack() as top:
        persist = top.enter_context(tc.tile_pool(name="persist", bufs=1))
        # QT rows 0..511 (tiles 0-3, head pair t on tile t), KT rows 512..1023
        qkt = [persist.tile([128, S], F16, name=f"qkt{m}") for m in range(8)]
        # V in (seq-part, head*dh free) orientation, 16 seq tiles
        vsb = [persist.tile([128, HL * (DH + 1)], F16, name=f"vsb{m}") for m in range(16)]
        # attn-out^T (head*dh on partitions, seq free); written unnormalized
        # by the PSUM drains, then normalized in place
        aot = [persist.tile([128, S], F16, name=f"aot{t}") for t in range(4)]
        cm = persist.tile([128, 4 * 512], F16, name="cm")
        ones_bc = persist.tile([65, 64], F16, name="ones_bc")
        nc.vector.memset(ones_bc, 1.0)
        eps_t = persist.tile([128, 1], F32, name="eps_t")
        nc.vector.memset(eps_t, LN_EPS)
        for m in range(16):
            vones = vsb[m].rearrange("p (h c) -> p h c", c=DH + 1)[:, :, DH:DH + 1]
            nc.vector.memset(vones, 1.0)
        nc.sync.dma_start(out=cm, in_=cmsk[:, :])

        # pool creation order is LIFO per memory space: fin pools (live to
        # the end) first, then attention pools, then projection pools last so
        # proj_ctx can close mid-program and free its SBUF/PSUM for fps.
        fin = ExitStack()
        dpool = fin.enter_context(tc.tile_pool(name="dram", bufs=1, space="DRAM"))
        fsb = fin.enter_context(tc.tile_pool(name="fsb", bufs=1))
        lnp = fin.enter_context(tc.tile_pool(name="lnp", bufs=2))
        wos = [fsb.tile([128, D], F16, name=f"wos{k}") for k in range(4)]
        xr = [fsb.tile([128, D], F16, name=f"xr{k}") for k in range(8)]

        attn_ctx = ExitStack()
        adram = attn_ctx.enter_context(tc.tile_pool(name="adram", bufs=2, space="DRAM"))
        asb = attn_ctx.enter_context(tc.tile_pool(name="asb", bufs=4))
        scps = attn_ctx.enter_context(tc.tile_pool(name="scps", bufs=2, space="PSUM"))
        accps = attn_ctx.enter_context(tc.tile_pool(name="accps", bufs=1, space="PSUM"))
        small = attn_ctx.enter_context(tc.tile_pool(name="small", bufs=2))

        # ---- input DMAs: weights first, then x by seq chunks so the
        # projection can start after ~1/4 of x has landed; wo/xres last
        proj_ctx = ExitStack()
        proj_in = proj_ctx.enter_context(tc.tile_pool(name="proj_in", bufs=1))
        pjps = proj_ctx.enter_context(tc.tile_pool(name="pjps", bufs=2, space="PSUM"))
        xt = [proj_in.tile([128, S], F16, name=f"xt{k}") for k in range(8)]
        wqs = [proj_in.tile([128, 2 * HL * DH], F16, name=f"wqs{k}") for k in range(8)]
        wvs = [proj_in.tile([128, HL * DH], F16, name=f"wvs{k}") for k in range(8)]
        for k in range(8):
            nc.sync.dma_start(out=wvs[k], in_=wv[k * 128:(k + 1) * 128, :])
        for k in range(8):
            nc.sync.dma_start(out=xt[k][:, 0:512], in_=xT[k * 128:(k + 1) * 128, 0:512])
        for k in range(8):
            nc.sync.dma_start(out=wqs[k], in_=wqk[k * 128:(k + 1) * 128, :])
        for k in range(8):
            nc.sync.dma_start(out=xt[k][:, 512:2048], in_=xT[k * 128:(k + 1) * 128, 512:2048])
        for k in range(4):
            nc.sync.dma_start(out=wos[k], in_=wo[k * 128:(k + 1) * 128, :])
        for k in range(8):
            nc.sync.dma_start(out=xr[k], in_=xres[k * 128:(k + 1) * 128, :])

        # tiny AllReduce as a cross-core barrier: absorbs NEFF launch skew
        # between pair cores while the input DMAs stream, so the real
        # ReduceScatters later don't each pay a ~30us peer-wait
        bar_in = dpool.tile([2, 4], F16, name="bar_in", bufs=1)
        bar_out = dpool.tile([2, 4], F16, name="bar_out", bufs=1)
        nc.sync.dma_start(out=bar_in.opt(), in_=cmsk[0:2, 0:4])
        nc.gpsimd.collective_compute(
            "AllReduce", mybir.AluOpType.add,
            replica_groups=[[0, 1], [2, 3], [4, 5], [6, 7]],
            ins=[bar_in.opt()], outs=[bar_out.opt()])

        def proj_v(m):
            ps = pjps.tile([128, 512], F32, tag="pj", name="pjv")
            for k in range(8):
                nc.tensor.matmul(ps, xt[k][:, m * 128:(m + 1) * 128], wvs[k],
                                 start=(k == 0), stop=(k == 7))
            vdst = vsb[m].rearrange("p (h c) -> p h c", c=DH + 1)[:, :, 0:DH]
            nc.vector.tensor_copy(vdst, ps.rearrange("p (h c) -> p h c", c=DH))

        def proj_qk(m):
            for q4 in range(4):
                ps = pjps.tile([128, 512], F32, tag="pj", name="pjqk")
                for k in range(8):
                    nc.tensor.matmul(ps, wqs[k][:, m * 128:(m + 1) * 128],
                                     xt[k][:, q4 * 512:(q4 + 1) * 512],
                                     start=(k == 0), stop=(k == 7))
                nc.vector.tensor_copy(qkt[m][:, q4 * 512:(q4 + 1) * 512], ps)

        def attn_chunk(t, qc):
            q_t, k_t = qkt[t], qkt[4 + t]
            nkb = 4 * qc + 4
            qsl = slice(qc * 512, (qc + 1) * 512)
            av0 = accps.tile([65, 512], F32, tag="av0", name="av0")
            av1 = accps.tile([65, 512], F32, tag="av1", name="av1")
            w = DH + 1

            def scores(kb):
                # both heads' scores in one 2-bank PSUM tile: one merged exp.
                # The two K=64 matmuls row-tile into PE quadrants (0,0)/(64,0).
                ksl = slice(kb * 128, (kb + 1) * 128)
                s01 = scps.tile([128, 1024], F32, tag="s01", name="s01")
                nc.tensor.matmul(s01[:, 0:512], k_t[0:64, ksl], q_t[0:64, qsl],
                                 start=True, stop=True)
                nc.tensor.matmul(s01[:, 512:1024], k_t[64:128, ksl], q_t[64:128, qsl],
                                 start=True, stop=True)
                e01 = asb.tile([128, 1024], F16, tag="e01", name="e01", bufs=4)
                nc.scalar.activation(e01, s01, Exp, scale=SCALE)
                r = kb - 4 * qc
                qoff = r * 128 if r > 0 else 0
                if r >= 0:  # diagonal block: zero masked entries (valid cols only)
                    msl = slice(r * 512 + qoff, (r + 1) * 512)
                    nc.vector.tensor_mul(e01[:, qoff:512], e01[:, qoff:512], cm[:, msl])
                    nc.vector.tensor_mul(e01[:, 512 + qoff:1024],
                                         e01[:, 512 + qoff:1024], cm[:, msl])
                return e01, qoff

            def av(kb, e01, qoff, st, sp):
                # attn-out^T accumulation; V carries a trailing ones column,
                # so the softmax denominator accumulates into PSUM row 64.
                # Diagonal blocks only contribute to queries >= qoff.
                nc.tensor.matmul(av0[:, qoff:512], vsb[kb][:, (2 * t) * w:(2 * t + 1) * w],
                                 e01[:, qoff:512], start=st, stop=sp)
                nc.tensor.matmul(av1[:, qoff:512], vsb[kb][:, (2 * t + 1) * w:(2 * t + 2) * w],
                                 e01[:, 512 + qoff:1024], start=st, stop=sp)

            # key blocks in pairs: both blocks' score matmuls (64-row PE mode)
            # run back-to-back, then both av matmuls (128-row mode) — halves
            # the PE tiling-mode switches, which drain the array
            for kb0 in range(0, nkb, 2):
                e_a, qo_a = scores(kb0)
                e_b, qo_b = scores(kb0 + 1)
                av(kb0, e_a, qo_a, kb0 == 0, False)
                av(kb0 + 1, e_b, qo_b, False, kb0 + 1 == nkb - 1)
            # drain PSUM (unnormalized, fp16). Denominator rows are copied to
            # SBUF, then a K=1 ones-matmul replicates each across 64 PSUM
            # partitions (no DRAM roundtrip), fast-reciprocal, normalize.
            d0 = small.tile([65, 512], F16, tag="d0", name="d0")
            d1 = small.tile([65, 512], F16, tag="d1", name="d1")
            nc.vector.tensor_copy(d0[64:65, :], av0[64:65, :])
            nc.vector.tensor_copy(d1[64:65, :], av1[64:65, :])
            nc.vector.tensor_copy(aot[t][0:64, qsl], av0[0:64, :])
            stg = small.tile([64, 512], F16, tag="stg", name="stg", bufs=4)
            nc.vector.tensor_copy(stg, av1[0:64, :])
            nc.sync.dma_start(out=aot[t][64:128, qsl], in_=stg)
            if fps is None:  # qc0 sweep: out-proj PSUM not allocated yet
                rbps = pjps.tile([128, 512], F32, tag="pj", name="rbps")
            else:
                rbps = fps.tile([128, 1024], F32, tag="po", name="rbps")
            nc.tensor.matmul(rbps[0:64, 0:512], ones_bc[64:65, :], d0[64:65, :],
                             start=True, stop=True)
            nc.tensor.matmul(rbps[64:128, 0:512], ones_bc[64:65, :], d1[64:65, :],
                             start=True, stop=True)
            rb = small.tile([128, 512], F32, tag="rb", name="rb")
            nc.vector.reciprocal_approx_fast(rb, rbps[:, 0:512])
            nc.vector.tensor_mul(aot[t][:, qsl], aot[t][:, qsl], rb)

        # fps (PSUM for the output projection) is allocated after proj_ctx
        # closes so it can reuse pjps's banks (8-bank budget).
        fps = None
        # per-pair-chunk fp16 ReduceScatter: chunk c carries output rows
        # [qc*512, qc*512+512); even core keeps the first 256, odd the last.
        rs_in = [dpool.tile([512, D], F16, name=f"rs_in{c}", bufs=1) for c in range(4)]
        rs_out = [dpool.tile([256, D], F16, name=f"rs_out{c}", bufs=1) for c in range(4)]

        rsq_magic = persist.tile([128, 1], mybir.dt.int32, name="rsq_magic")
        nc.vector.memset(rsq_magic.bitcast(F32), np.uint32(0x5F3759DF).view(np.float32).item())

        def _rsqrt_dve(v, out):
            """out = (v+eps)^-0.5 via quake seed + 2 Newton-Raphson steps."""
            ve = lnp.tile([128, 1], F32, tag="rq_ve", name="rq_ve")
            nc.vector.tensor_scalar(out=ve, in0=v, scalar1=LN_EPS, scalar2=None,
                                    op0=mybir.AluOpType.add)
            ri = lnp.tile([128, 1], mybir.dt.int32, tag="rq_ri", name="rq_ri")
            nc.vector.tensor_scalar(out=ri, in0=ve.bitcast(mybir.dt.int32), scalar1=1,
                                    scalar2=None, op0=mybir.AluOpType.logical_shift_right)
            nc.vector.tensor_tensor(out=ri, in0=rsq_magic, in1=ri,
                                    op=mybir.AluOpType.subtract)
            r = ri.bitcast(F32)
            hv = lnp.tile([128, 1], F32, tag="rq_hv", name="rq_hv")
            nc.vector.tensor_scalar(out=hv, in0=ve, scalar1=0.5, scalar2=None,
                                    op0=mybir.AluOpType.mult)
            t1 = lnp.tile([128, 1], F32, tag="rq_t1", name="rq_t1")
            for it in range(2):
                nc.vector.tensor_tensor(out=t1, in0=r, in1=r, op=mybir.AluOpType.mult)
                nc.vector.tensor_tensor(out=t1, in0=t1, in1=hv, op=mybir.AluOpType.mult)
                nc.vector.tensor_scalar(out=t1, in0=t1, scalar1=-1.0, scalar2=1.5,
                                        op0=mybir.AluOpType.mult, op1=mybir.AluOpType.add)
                dst = out if it == 1 else r
                nc.vector.tensor_tensor(out=dst, in0=r, in1=t1, op=mybir.AluOpType.mult)

        def out_chunk(c):
            qc = CHUNK_QC[c]
            for j in range(4):
                g = qc * 4 + j
                pstg = lnp.tile([128, D], F16, tag="pstg", name="pstg")
                po = fps.tile([128, 1024], F32, tag="po", name="po")
                for n2 in range(2):
                    for k in range(4):
                        nc.tensor.matmul(po[:, n2 * 512:(n2 + 1) * 512],
                                         aot[k][:, g * 128:(g + 1) * 128],
                                         wos[k][:, n2 * 512:(n2 + 1) * 512],
                                         start=(k == 0), stop=(k == 3))
                nc.vector.tensor_copy(pstg, po)
                nc.sync.dma_start(out=rs_in[c][j * 128:(j + 1) * 128, :], in_=pstg)
            nc.gpsimd.collective_compute(
                "ReduceScatter", mybir.AluOpType.add,
                replica_groups=[[0, 1], [2, 3], [4, 5], [6, 7]],
                ins=[rs_in[c].opt()], outs=[rs_out[c].opt()])

        def ln_chunk(c):
          with tc.high_priority(offset=-2000):
            for j in range(2):
                m = 2 * c + j
                yin = lnp.tile([128, D], F16, tag="yin", name="yin")
                nc.gpsimd.dma_start(out=yin, in_=rs_out[c][j * 128:(j + 1) * 128, :])
                y = lnp.tile([128, D], F16, tag="y", name="y")
                nc.vector.tensor_add(y, yin, xr[m])
                stats = lnp.tile([128, 2, 6], F32, tag="st", name="st")
                mv = lnp.tile([128, 2], F32, tag="mv", name="mv")
                for sg in range(2):
                    nc.vector.bn_stats(out=stats[:, sg, :], in_=y[:, sg * 512:(sg + 1) * 512])
                nc.vector.bn_aggr(out=mv, in_=stats)
                # rstd = (var+eps)^-0.5 on DVE (magic-constant seed + 2
                # Newton steps) so ACT only ever needs the exp table set
                rstd = lnp.tile([128, 1], F32, tag="rs", name="rs")
                _rsqrt_dve(mv[:, 1:2], rstd)
                ot = lnp.tile([128, D], F16, tag="ot", name="ot")
                nc.vector.tensor_scalar(out=ot, in0=y, scalar1=mv[:, 0:1], scalar2=rstd,
                                        op0=mybir.AluOpType.subtract,
                                        op1=mybir.AluOpType.mult)
                nc.sync.dma_start(out=out[m * 128:(m + 1) * 128, :], in_=ot)

        # ---- schedule: projection interleaved into the first (qc=0) sweep;
        # each RS fires right after its sweep so it hides under the next one.
        for t in range(4):
            proj_v(4 * t)
            proj_v(4 * t + 1)
            proj_v(4 * t + 2)
            proj_v(4 * t + 3)
            proj_qk(t)
            proj_qk(4 + t)
            attn_chunk(t, 0)
        proj_ctx.close()
        fps_ctx = ExitStack()
        fps = fps_ctx.enter_context(tc.tile_pool(name="fps", bufs=1, space="PSUM"))
        out_chunk(0)
        for t in range(4):
            attn_chunk(t, 2)
        out_chunk(1)
        for t in range(4):
            attn_chunk(t, 1)
        out_chunk(2)
        ln_chunk(0)
        for t in range(4):
            attn_chunk(t, 3)
        out_chunk(3)
        ln_chunk(1)
        ln_chunk(2)
        ln_chunk(3)
        fps_ctx.close()
        attn_ctx.close()
        fin.close()
    nc.compile()
    return nc


def _build_cmask():
    k = np.arange(128)[:, None]
    q = np.arange(512)[None, :]
    blocks = [(r * 128 + k <= q).astype(np.float16) for r in range(4)]
    return np.concatenate(blocks, axis=1)


def _dr8(a):
    """[1024, N] -> DoubleRow-interleaved fp8e4m3 [128, 8, N]."""
    import ml_dtypes
    n = a.shape[1]
    return np.ascontiguousarray(
        np.asarray(a, np.float32).reshape(8, 128, n).transpose(1, 0, 2)
    ).astype(ml_dtypes.float8_e4m3)


def _core_rows(half):
    """Absolute query rows this core outputs, in out-tile order."""
    rows = []
    for c in range(4):
        qbase = CHUNK_QC[c] * 512 + half * 256
        rows.append(np.arange(qbase, qbase + 256))
    return np.concatenate(rows)


def _make_in_maps(x0, W_in, W_o):
    x0 = np.asarray(x0, np.float32)
    W_in = np.asarray(W_in, np.float32)
    W_o = np.asarray(W_o, np.float32)
    wo16 = W_o.astype(np.float16)
    cmask = _build_cmask()
    in_maps = []
    for core in range(NCORES):
        bi, half = core // 2, core % 2
        hs = range(half * HL, half * HL + HL)
        wqk = np.concatenate(
            [W_in[:, h * 3 * DH: h * 3 * DH + DH] for h in hs]
            + [W_in[:, h * 3 * DH + DH: h * 3 * DH + 2 * DH] for h in hs], axis=1)
        wv = np.concatenate(
            [W_in[:, h * 3 * DH + 2 * DH: h * 3 * DH + 3 * DH] for h in hs], axis=1)
        rows = _core_rows(half)
        in_maps.append(dict(
            xT=np.ascontiguousarray(x0[bi].T).astype(np.float16),
            wqk=np.ascontiguousarray(wqk).astype(np.float16),
            wv=np.ascontiguousarray(wv).astype(np.float16),
            wo=np.ascontiguousarray(wo16[half * HL * DH:(half + 1) * HL * DH]),
            xres=np.ascontiguousarray(x0[bi][rows]).astype(np.float16),
            cmask=cmask))
    return in_maps


_NC = None


def _run(x0, W_in, W_o, **run_kwargs):
    global _NC
    if _NC is None:
        _NC = build_nc()
    in_maps = _make_in_maps(x0, W_in, W_o)
    return run_bass_kernel_spmd(_NC, in_maps, list(range(NCORES)), **run_kwargs)


def kernel(x0, W_in, W_o, src_mask=None):
    res = _run(x0, W_in, W_o).results
    out = np.empty((B, S, D), np.float32)
    for core in range(NCORES):
        bi, half = core // 2, core % 2
        out[bi][_core_rows(half)] = res[core]["out"].astype(np.float32)
    return out
